# revision 4
# baseline (speedup 1.0000x reference)
"""Trainium2 Bass kernel: single-layer GRU (T=512, B=64, F=128, H=512) + output proj (O=16).

Sharding: data-parallel over batch. B=64 -> 8 cores x 8 sequences each.
Weights replicated; the recurrence is fully local per core.

The recurrence is PE-stream-bound in principle (48 LDWEIGHTS+MATMUL pairs per
step at ~32ns sustained = ~1.6us/step), but a naive schedule stalls on the
elementwise chain that turns PSUM gate pre-activations into h(t) (~1.2us
latency). This kernel hides the chain with a "z-last" structure:

  h(t) = n + z * (h(t-1) - n)        [algebraically (1-z)n + z h]

Per h-half, gates are split into TWO PSUM groups:
  ps_rn [128, (r0,r1,n0,n1), 8]  - closes EARLY-MID pass. Its chain
        (sigmoid r -> mul -> add xn -> tanh n -> d = h_prev - n) runs while
        the z matmuls still stream on the PE.
  ps_z  [128, (z0,z1), 8]        - closes LAST. Its post-close chain is only
        sigmoid(z) -> m = z*d -> h = n + m  (~0.6us), so h(t) lands early in
        pass t+1 and the PE barely waits.

Gate-side x contributions (and biases) are precomputed as xg and injected
into PSUM via identity matmuls (no dependency on h -> scheduled early); the
n-columns of ps_rn are opened with a broadcast b_hn injection so the n-gate
chain needs no per-partition-scalar stt ops.

Device W_hh column order: [r0,r1,n0,n1 | r2,r3,n2,n3 | z0,z1 | z2,z3]
xg column order:          [r0,r1,z0,z1,n0,n1 | r2,r3,z2,z3,n2,n3]

build_nc(reps=N) wraps the body in a tc.For_i hardware loop for the
reps-differencing timing method used by test.py.
"""

import numpy as np
import ml_dtypes
from contextlib import ExitStack

import concourse.bass as bass
import concourse.tile as tile
from concourse import bacc, mybir
from concourse.bass import ds, ts
from concourse.bass_utils import run_bass_kernel_spmd

T, B, F, H, O = 512, 64, 128, 512, 16
N_CORES = 8
BL = B // N_CORES          # 8 sequences per core
GC = (3 * H) // 128        # 12 gate chunks
HC = H // 128              # 4 hidden chunks
TCH = 8                    # xg is staged in 8 chunks of 64 timesteps
TC = T // TCH              # 64

# original gate-chunk indices: r=0..3, z=4..7, n=8..11
PERM_W = [0, 1, 8, 9, 2, 3, 10, 11, 4, 5, 6, 7]   # W_hh device column blocks
PERM_X = [0, 1, 4, 5, 8, 9, 2, 3, 6, 7, 10, 11]   # xg blocks (r,z,n per half)

F32 = mybir.dt.float32
BF16 = mybir.dt.bfloat16
BF_NP = ml_dtypes.bfloat16

WSCALE = 1.0
INV_WSCALE = 1.0 / WSCALE


def build_nc(t_steps: int = T, reps: int = 1):
    """Build + compile the per-core Bass program (SPMD: same program, 8 cores)."""
    FT = mybir.ActivationFunctionType
    nc = bacc.Bacc("TRN2", target_bir_lowering=False, debug=False,
                   num_devices=N_CORES)

    x_in = nc.dram_tensor("x", [128, T * BL], BF16, kind="ExternalInput")
    whh_in = nc.dram_tensor("w_hh_t", [HC, 128, GC * 128], BF16, kind="ExternalInput")
    wih_in = nc.dram_tensor("w_ih_t", [128, GC * 128], BF16, kind="ExternalInput")
    bias_in = nc.dram_tensor("biasg", [128, GC], F32, kind="ExternalInput")
    bhn_in = nc.dram_tensor("bhn_bc", [128, HC * BL], BF16, kind="ExternalInput")
    wout_in = nc.dram_tensor("w_out_t", [HC, 128, O], BF16, kind="ExternalInput")
    bout_in = nc.dram_tensor("b_out_p", [O, 1], F32, kind="ExternalInput")
    ident_in = nc.dram_tensor("ident", [128, 128], BF16, kind="ExternalInput")
    y_out = nc.dram_tensor("y", [O, T * BL], F32, kind="ExternalOutput")

    with tile.TileContext(nc) as tc, ExitStack() as ctx:
        const = ctx.enter_context(tc.tile_pool(name="const", bufs=1))
        psum = ctx.enter_context(tc.tile_pool(name="psum", bufs=2, space="PSUM"))
        work = ctx.enter_context(tc.tile_pool(name="work", bufs=2))

        # ---- constants / inputs to SBUF
        x_sb = const.tile([128, T * BL], BF16)
        nc.sync.dma_start(x_sb[:], x_in.ap()[:])
        whh_sb = const.tile([128, HC, GC * 128], BF16)
        for hc in range(HC):
            nc.sync.dma_start(whh_sb[:, hc, :], whh_in.ap()[hc])
        wih_sb = const.tile([128, GC * 128], BF16)
        nc.sync.dma_start(wih_sb[:], wih_in.ap()[:])
        bias_sb = const.tile([128, GC], F32)
        nc.sync.dma_start(bias_sb[:], bias_in.ap()[:])
        bhn_sb = const.tile([128, HC, BL], BF16)
        nc.sync.dma_start(bhn_sb[:], bhn_in.ap()[:])
        wout_sb = const.tile([128, HC, O], BF16)
        for hc in range(HC):
            nc.sync.dma_start(wout_sb[:, hc, :], wout_in.ap()[hc])
        bout_sb = const.tile([O, 1], F32)
        nc.sync.dma_start(bout_sb[:], bout_in.ap()[:])
        ident_sb = const.tile([128, 128], BF16)
        nc.sync.dma_start(ident_sb[:], ident_in.ap()[:])

        hs_sb = const.tile([128, T, HC, BL], BF16)
        h0_bf = const.tile([128, HC, BL], BF16)
        nc.vector.memset(h0_bf[:], 0)
        xg_tiles = [const.tile([128, GC, TC * BL], BF16, name=f"xg{i}")
                    for i in range(TCH)]

        # xg block offsets per half: r at 6h+0:2, z at 6h+2:4, n at 6h+4:6
        def xr(xg, half, tb):
            return xg[:, 6 * half + 0:6 * half + 2, tb]

        def xz(xg, half, tb):
            return xg[:, 6 * half + 2:6 * half + 4, tb]

        def xn(xg, half, tb):
            return xg[:, 6 * half + 4:6 * half + 6, tb]

        def body():
            # ---- phase 1: xg = w_ih' . x + biasg (device xg order)
            for c in range(TCH):
                for g in range(GC):
                    ps = psum.tile([128, TC * BL], F32, tag=f"p{g % 4}")
                    nc.tensor.matmul(ps[:], wih_sb[:, ts(g, 128)],
                                     x_sb[:, ts(c, TC * BL)], start=True, stop=True)
                    dst = xg_tiles[c][:, g, :]
                    if g % 2 == 0:
                        nc.scalar.activation(dst, ps[:], FT.Identity,
                                             bias=bias_sb[:, g:g + 1], scale=1.0)
                    else:
                        nc.vector.tensor_scalar_add(dst, ps[:], bias_sb[:, g:g + 1])

            # ---- phase 2: the recurrence
            for t in range(t_steps):
                c, tt = divmod(t, TC)
                xg = xg_tiles[c]
                tb = ds(tt * BL, BL)
                rhs = h0_bf if t == 0 else hs_sb[:, t - 1, :, :]

                ps_rn = [psum.tile([128, 4, BL], F32, tag="p0", name="ps_rn0"),
                         psum.tile([128, 4, BL], F32, tag="p2", name="ps_rn1")]
                ps_z = [psum.tile([128, 2, BL], F32, tag="p1", name="ps_z0"),
                        psum.tile([128, 2, BL], F32, tag="p3", name="ps_z1")]

                # open rn groups: inject xr (cols 0:2) + b_hn broadcast (cols 2:4)
                for half in (0, 1):
                    nc.tensor.matmul(ps_rn[half][:, 0:2, :], ident_sb[:],
                                     xr(xg, half, tb),
                                     start=True, stop=False, skip_group_check=True)
                    nc.tensor.matmul(ps_rn[half][:, 2:4, :], ident_sb[:],
                                     bhn_sb[:, 2 * half:2 * half + 2, :],
                                     start=False, stop=False, skip_group_check=True)

                def rn_mm(half, j, hc, stop=False):
                    col = 4 * half + j        # W device col block in rn region
                    nc.tensor.matmul(
                        ps_rn[half][:, j, :],
                        whh_sb[:, hc, ds(col * 128, 128)],
                        rhs[:, hc, :],
                        start=False, stop=stop, skip_group_check=True)

                def z_mm(half, j, hc, stop=False):
                    col = 8 + 2 * half + j    # z region starts at block 8
                    nc.tensor.matmul(
                        ps_z[half][:, j, :],
                        whh_sb[:, hc, ds(col * 128, 128)],
                        rhs[:, hc, :],
                        start=False, stop=stop, skip_group_check=True)

                # rn matmuls: h-half0 consumers (hc 0,1) first, then hc 2,3;
                # close ps_rn[0] before ps_rn[1].
                for hc in (0, 1):
                    for half in (0, 1):
                        for j in range(4):
                            rn_mm(half, j, hc)
                for half in (0, 1):
                    for hc in (2, 3):
                        for j in range(4):
                            rn_mm(half, j, hc, stop=(hc == 3 and j == 3))

                # z groups open late (inject xz), then z matmuls; ps_z[0]
                # closes before ps_z[1].
                for half in (0, 1):
                    nc.tensor.matmul(ps_z[half][:], ident_sb[:],
                                     xz(xg, half, tb),
                                     start=True, stop=False, skip_group_check=True)
                for half in (0, 1):
                    for j in range(2):
                        for hc in range(HC):
                            z_mm(half, j, hc, stop=(j == 1 and hc == 3))

                # elementwise: rn chain is long but overlaps the z matmuls;
                # z post-close chain is short: sigmoid -> mul -> add.
                # Emission is stage-interleaved across halves so each
                # engine's IN-ORDER queue matches data readiness (a
                # late-input op emitted early would block the queue).
                # Engine split: ACT rs/nt/zs, DVE nm/np, Pool d/m/h (the
                # tail rides Pool's in-order queue with no sem hops).
                rs, nm, np_, nt, d, zs, m = ({} for _ in range(7))
                for half in (0, 1):
                    rs[half] = work.tile([128, 2, BL], F32, tag=f"rs{half}",
                                         name=f"rs{half}")
                    nc.scalar.activation(rs[half][:], ps_rn[half][:, 0:2, :],
                                         FT.Sigmoid, scale=INV_WSCALE)
                for half in (0, 1):
                    nm[half] = work.tile([128, 2, BL], F32, tag=f"nm{half}",
                                         name=f"nm{half}")
                    nc.vector.tensor_mul(nm[half][:], ps_rn[half][:, 2:4, :],
                                         rs[half][:])
                for half in (0, 1):
                    np_[half] = work.tile([128, 2, BL], F32, tag=f"np{half}",
                                          name=f"np{half}")
                    nc.vector.tensor_add(np_[half][:], nm[half][:],
                                         xn(xg, half, tb))
                for half in (0, 1):
                    nt[half] = work.tile([128, 2, BL], F32, tag=f"nt{half}",
                                         name=f"nt{half}")
                    nc.scalar.activation(nt[half][:], np_[half][:], FT.Tanh,
                                         scale=INV_WSCALE)
                for half in (0, 1):
                    d[half] = work.tile([128, 2, BL], F32, tag=f"d{half}",
                                        name=f"d{half}")
                    nc.gpsimd.tensor_sub(d[half][:],
                                         rhs[:, ds(2 * half, 2), :],
                                         nt[half][:])
                for half in (0, 1):
                    zs[half] = work.tile([128, 2, BL], F32, tag=f"zs{half}",
                                         name=f"zs{half}")
                    nc.scalar.activation(zs[half][:], ps_z[half][:],
                                         FT.Sigmoid, scale=INV_WSCALE)
                for half in (0, 1):
                    m[half] = work.tile([128, 2, BL], F32, tag=f"m{half}",
                                        name=f"m{half}")
                    nc.gpsimd.tensor_mul(m[half][:], zs[half][:], d[half][:])
                    nc.gpsimd.tensor_add(hs_sb[:, t, ds(2 * half, 2), :],
                                         nt[half][:], m[half][:])

            # ---- phase 3: y = w_out . h_t + b_out
            for c in range(TCH):
                ps = psum.tile([O, TC * BL], F32, tag="p0")
                for hc in range(HC):
                    nc.tensor.matmul(ps[:], wout_sb[:, hc, :],
                                     hs_sb[:, ts(c, TC), hc, :],
                                     start=(hc == 0), stop=(hc == 3))
                yt = work.tile([O, TC * BL], F32, tag="yt")
                nc.scalar.activation(yt[:], ps[:], FT.Identity, bias=bout_sb[:],
                                     scale=1.0)
                nc.sync.dma_start(y_out.ap()[:, ts(c, TC * BL)], yt[:])

        if reps == 1:
            body()
        else:
            with tc.For_i(0, reps):
                body()

    nc.compile()
    return nc


def prep_inputs(x_rnn, w_ih, w_hh, b_ih, b_hh, w_out, b_out):
    """Host-side shard + relayout. Returns per-core in_maps."""
    x_rnn = np.asarray(x_rnn, np.float32)
    w_ih = np.asarray(w_ih, np.float32)
    w_hh = np.asarray(w_hh, np.float32)
    b_ih = np.asarray(b_ih, np.float32)
    b_hh = np.asarray(b_hh, np.float32)
    w_out = np.asarray(w_out, np.float32)
    b_out = np.asarray(b_out, np.float32)

    rows_w = np.concatenate([np.arange(b * 128, (b + 1) * 128) for b in PERM_W])
    rows_x = np.concatenate([np.arange(b * 128, (b + 1) * 128) for b in PERM_X])
    w_hh_p = w_hh[rows_w]                     # (1536, 512), device W order
    w_ih_p = w_ih[rows_x]                     # (1536, 128), device xg order
    # xg biases: r,z chunks carry b_ih+b_hh; n chunks carry b_ih only
    # (b_hn is injected into PSUM separately, inside the r* product).
    bsum = (b_ih + b_hh)[rows_x]
    n_pos = [i for i, b in enumerate(PERM_X) if b >= 8]
    bih_p = b_ih[rows_x]
    for p in n_pos:
        bsum[p * 128:(p + 1) * 128] = bih_p[p * 128:(p + 1) * 128]
    biasg = bsum.reshape(GC, 128).T.copy() * WSCALE             # (128, GC) f32

    bhn = b_hh[2 * H:].reshape(HC, 128).T * WSCALE              # (128, HC)
    bhn_bc = np.repeat(bhn[:, :, None], BL, axis=2).reshape(128, HC * BL)

    w_ih_t = np.ascontiguousarray(w_ih_p.T * WSCALE).astype(BF_NP)  # (128, 1536)
    w_hh_t = np.ascontiguousarray(
        w_hh_p.T.reshape(HC, 128, GC * 128) * WSCALE).astype(BF_NP)
    w_out_t = np.ascontiguousarray(w_out.T.reshape(HC, 128, O)).astype(BF_NP)
    b_out_p = b_out.reshape(O, 1).astype(np.float32)
    ident = np.eye(128, dtype=BF_NP)

    in_maps = []
    for c in range(N_CORES):
        xc = x_rnn[:, c * BL:(c + 1) * BL, :]             # (T, 8, 128)
        x_t = np.ascontiguousarray(xc.transpose(2, 0, 1).reshape(128, T * BL))
        in_maps.append({
            "x": x_t.astype(BF_NP),
            "w_hh_t": w_hh_t, "w_ih_t": w_ih_t, "biasg": biasg.astype(np.float32),
            "bhn_bc": bhn_bc.astype(BF_NP),
            "w_out_t": w_out_t, "b_out_p": b_out_p, "ident": ident,
        })
    return in_maps


def assemble_output(results):
    """results: list of per-core {"y": (O, T*BL)} -> full (T, B, O) f32."""
    ys = []
    for c in range(N_CORES):
        yc = np.asarray(results[c]["y"], np.float32)
        ys.append(yc.reshape(O, T, BL).transpose(1, 2, 0))
    return np.concatenate(ys, axis=1)


_NC_CACHE = {}


def get_nc(t_steps: int = T, reps: int = 1):
    if (t_steps, reps) not in _NC_CACHE:
        _NC_CACHE[(t_steps, reps)] = build_nc(t_steps, reps)
    return _NC_CACHE[(t_steps, reps)]


def kernel(**inputs) -> np.ndarray:
    nc = get_nc()
    in_maps = prep_inputs(**inputs)
    res = run_bass_kernel_spmd(nc, in_maps, list(range(N_CORES)))
    return assemble_output(res.results)
